# revision 27
# baseline (speedup 1.0000x reference)
"""GCN layer kernel for TRN2, data-parallel over batch across 8 NeuronCores.

The two matmuls of the layer compose linearly (the relu sits after both):
  out2[l,o] = dis_l * sum_j aT[j,l] * dis_j * (x @ W.T)[j,o]
so the device computes XW = x @ W.T first (bf16, from a host-transposed x,
during the adjacency DMA stream), then one big fp8 DoubleRow matmul against
the mask+self-loop-folded adjacency (host-prepared, fp8, paired-row layout
[128, 2, L] per j-tile) lands the pre-relu directly in [l, o] layout.

Phases:
  H: DMA wt/xT (bf16), then aT (fp8), then x (f32). PE: XW quarters
     (m-major, 3 PSUM banks) interleaved 3-matmuls-per-tile with the deg
     row-sum matvecs (fp8 DoubleRow vs a 16-wide ones stationary) so the PE
     rides the aT stream without idling.
  T: deg rows -> column layout via PE transposes, sqrt off PSUM,
     reciprocal; y = 16 * dis * XW cast to fp8 (the 16 keeps fp8 out of
     subnormals; 1/16 rides the relu scale).
  M: per 128-row block: 8 DoubleRow accumulation matmuls (j pairs), relu
     straight off PSUM with per-partition scale dis_l/(16*sqrt(D)), bf16
     residual hh = relu + x, bn_stats/bn_aggr moments, normalize, stream
     out. The last block runs in o-halves to shorten the serial tail.

Precision: adjacency and y in fp8e4m3, XW/W in bf16, accumulation in fp32
PSUM, LN in fp32 with bf16 hh. Measured rel err ~2-5e-3 vs the fp32
reference (gate: 2e-2).
"""
import os
import numpy as np
import ml_dtypes

import concourse.bacc as bacc
import concourse.tile as tile
import concourse.mybir as mybir
from concourse.bass_utils import run_bass_kernel_spmd

B, L, D = 8, 2048, 512
JBN = L // 128      # 16 row blocks (j tiles / l blocks)
NCH = L // 512      # 4 deg psum chunks of 512
DBN = D // 128      # 4 d-blocks
LN_EPS = 1e-5
DSCALE = float(D) ** -0.5
F32 = mybir.dt.float32
BF16 = mybir.dt.bfloat16
FP8 = mybir.dt.float8e4
YSCALE = 16.0
MUL = mybir.AluOpType.mult
ADD = mybir.AluOpType.add
SUB = mybir.AluOpType.subtract

LAST_RESULT = None  # BassKernelResults of the most recent run (for profiling)


def _build_program(fold_scale=True, ln_identity=True):
    """fold_scale: edge_weight folded into W host-side and bias == 0, so the
    relu collapses to an activation with per-partition scale dis_l*DSCALE.
    ln_identity: ln_w == 1, ln_b == 0."""
    nc = bacc.Bacc("TRN2", target_bir_lowering=False, debug=False)
    d = {}
    def di(name, shape, dt):
        d[name] = nc.dram_tensor(name, shape, dt, kind="ExternalInput").ap()
    di("ident", [128, 128], F32)
    di("aT_p8", [L // 2, 2 * L], FP8)
    di("xT_h", [D, L], BF16)
    di("x_in", [L, D], F32)
    di("wt_h", [D, D], BF16)
    if not fold_scale:
        di("ewc", [128, 1], F32)
        di("b_row", [1, D], F32)
    if not ln_identity:
        di("lnw_row", [1, D], F32)
        di("lnb_row", [1, D], F32)
    out_d = nc.dram_tensor("out_t", [L, D], F32, kind="ExternalOutput").ap()

    with tile.TileContext(nc) as tc:
        with tc.tile_pool(name="pA", bufs=JBN) as pA, \
             tc.tile_pool(name="pX", bufs=JBN) as pX, \
             tc.tile_pool(name="pY", bufs=JBN) as pY, \
             tc.tile_pool(name="pXW", bufs=JBN) as pXW, \
             tc.tile_pool(name="pW", bufs=DBN) as pW, \
             tc.tile_pool(name="pCol", bufs=12) as pCol, \
             tc.tile_pool(name="pSmall", bufs=1) as pSmall:

            # ---- persistent arrays ----
            aT_t = [pA.tile([128, 2, L], FP8, tag="aT", name=f"aT{j}")
                    for j in range(JBN // 2)]
            x_t = [pX.tile([128, D], F32, tag="x", name=f"x{j}") for j in range(JBN)]
            xw_t = [pXW.tile([128, D], BF16, tag="xw", name=f"xw{j}")
                    for j in range(JBN)]
            pXTh_cm = tc.tile_pool(name="pXTh", bufs=DBN)
            pXTh = pXTh_cm.__enter__()
            xTh_t = [pXTh.tile([128, L], BF16, tag="xTh", name=f"xTh{m}")
                     for m in range(DBN)]
            y_t = [pY.tile([128, 2, D], FP8, tag="y", name=f"y{j}")
                   for j in range(JBN // 2)]
            wt_t = [pW.tile([128, D], BF16, tag="wt", name=f"wt{k}")
                    for k in range(DBN)]
            eps_t = pSmall.tile([128, 1], F32, tag="eps")
            nc.vector.memset(eps_t[:], LN_EPS)
            # dual-fp8 ldweights needs 16B-aligned kk stride: 16 duplicate
            # ones columns; rows 1..15 of each deg psum output are ignored
            onesc_t = pSmall.tile([128, 2, 16], FP8, tag="onesc")
            nc.vector.memset(onesc_t[:], 1.0)
            # touch every activation function now so the Act table loads
            # happen during the DMA stream, not on the critical path later
            warm_t = pSmall.tile([128, 1], F32, tag="warm")
            nc.scalar.sqrt(warm_t[:], eps_t[:])
            nc.scalar.activation(warm_t[:], eps_t[:],
                                 mybir.ActivationFunctionType.Square)
            nc.scalar.activation(warm_t[:], eps_t[:],
                                 mybir.ActivationFunctionType.Relu)
            stat_b = {}

            # DMA order: ident, W, xT (for XW), aT (gates deg), x f32 last
            for k in range(DBN):
                nc.sync.dma_start(wt_t[k][:], d["wt_h"][k * 128:(k + 1) * 128, :])
                rsl = slice(k * 128, (k + 1) * 128)
                if k < 2:
                    # split: the first 512 columns feed XW quarter 0 sooner
                    nc.sync.dma_start(xTh_t[k][:, 0:512], d["xT_h"][rsl, 0:512])
                    nc.sync.dma_start(xTh_t[k][:, 512:L], d["xT_h"][rsl, 512:L])
                else:
                    nc.sync.dma_start(xTh_t[k][:], d["xT_h"][rsl, :])
            ident_t = pSmall.tile([128, 128], F32, tag="ident")
            nc.sync.dma_start(ident_t[:], d["ident"][:])
            for j2 in range(JBN // 2):
                nc.sync.dma_start(aT_t[j2][:],
                                  d["aT_p8"][j2 * 128:(j2 + 1) * 128, :])
            for jb in range(JBN):
                nc.sync.dma_start(x_t[jb][:], d["x_in"][jb * 128:(jb + 1) * 128, :])
            if not fold_scale:
                ewc_t = pSmall.tile([128, 1], F32, tag="ew")
                nc.scalar.dma_start(ewc_t[:], d["ewc"][:])
                b_r = pSmall.tile([1, D], F32, tag="brow")
                nc.scalar.dma_start(b_r[:], d["b_row"][:])
                bbT = pSmall.tile([128, D], F32, tag="bb")
                nc.gpsimd.partition_broadcast(bbT[:], b_r[:])
            if not ln_identity:
                rows = {}
                for nm in ("lnw_row", "lnb_row"):
                    r = pSmall.tile([1, D], F32, tag=nm, name=nm + "_t")
                    nc.scalar.dma_start(r[:], d[nm][:])
                    rows[nm] = r
                for nm in ("lnw_row", "lnb_row"):
                    t = pSmall.tile([128, D], F32, tag=nm + "b", name=nm + "_b")
                    nc.gpsimd.partition_broadcast(t[:], rows[nm][:])
                    stat_b[nm] = t

            # ---- phase H: XW = x @ W.T on PE from the host-transposed x,
            # m-major over jb-halves in 8 PSUM banks, while aT streams ----
            # XW emission is interleaved with the deg matvecs: quarter 0
            # runs before the aT stream lands; the remaining 48 XW matmuls
            # are doled out 3 per deg tile so the PE rides the stream with
            # no idle (per-tile slot: 4 deg matvecs + 3 XW matmuls ~= the
            # 1.46us tile DMA cadence).
            psXW_cm = tc.tile_pool(name="psXW", bufs=3, space="PSUM")
            psXW = psXW_cm.__enter__()

            def xw_instruction_stream():
                for q in range(4):
                    jbs = range(q * 4, q * 4 + 4)
                    xwp = {jb: psXW.tile([128, D], F32, tag="xwp",
                                         name=f"xwp{jb}") for jb in jbs}
                    for m in range(DBN):
                        for jb in jbs:
                            yield "mm", (lambda jb=jb, m=m, xwp=xwp:
                                nc.tensor.matmul(
                                    xwp[jb][:],
                                    xTh_t[m][:, jb * 128:(jb + 1) * 128],
                                    wt_t[m][:],
                                    start=(m == 0), stop=(m == DBN - 1)))
                    for jb in jbs:
                        if jb % 2 == 0:
                            yield "drain", (lambda jb=jb, xwp=xwp:
                                nc.vector.tensor_copy(xw_t[jb][:], xwp[jb][:]))
                        else:
                            yield "drain", (lambda jb=jb, xwp=xwp:
                                nc.scalar.copy(xw_t[jb][:], xwp[jb][:]))

            xw_stream = xw_instruction_stream()
            def emit_xw(k):
                n = 0
                for kind, op in xw_stream:
                    op()
                    if kind == "mm":
                        n += 1
                        if n >= k:
                            break

            emit_xw(20)  # quarter 0 + the head of quarter 1

            with tc.tile_pool(name="psDeg", bufs=4, space="PSUM") as psDeg, \
                 tc.tile_pool(name="psPT", bufs=1, space="PSUM") as psPT, \
                 tc.tile_pool(name="pTr", bufs=1) as pTr:
                # deg: 4 chunk accumulators, DoubleRow dst must start at
                # partition 0 so each chunk gets its own bank (row 0 used)
                deg_ps = [psDeg.tile([128, 512], F32, tag="deg",
                                     name=f"deg_ps{i}") for i in range(NCH)]
                for j2 in range(JBN // 2):
                    for n in range(NCH):
                        nc.tensor.matmul(
                            deg_ps[n][0:16, :],
                            onesc_t[:],
                            aT_t[j2][:, :, n * 512:(n + 1) * 512],
                            start=(j2 == 0), stop=(j2 == JBN // 2 - 1),
                            perf_mode=mybir.MatmulPerfMode.DoubleRow)
                    emit_xw(6)
                emit_xw(1000)  # drain any remaining XW work
                # ---- phase T: deg -> dis -> y, two-stage pipeline ----
                r_sb = pTr.tile([128, 1024], F32, tag="rsb")
                rc_ps = psPT.tile([128, JBN], F32, tag="rc")
                std_col = pCol.tile([128, JBN], F32, tag="stdc", bufs=1)
                dis_col = pCol.tile([128, JBN], F32, tag="disc", bufs=1)
                diss_col = pCol.tile([128, JBN], F32, tag="dissc", bufs=1)
                for t in range(2):
                    csl = slice(t * 8, t * 8 + 8)
                    eng_copy = (nc.vector.tensor_copy if t == 0
                                else nc.scalar.copy)
                    eng_copy(r_sb[0:1, t * 512:(t + 1) * 512],
                             deg_ps[2 * t][0:1, :])
                    eng_copy(r_sb[32:33, t * 512:(t + 1) * 512],
                             deg_ps[2 * t + 1][0:1, :])
                    for v in range(t * 8, t * 8 + 8):
                        n, c = v // 4, v % 4
                        po = 32 * (n % 2)
                        fo = (n // 2) * 512 + c * 128
                        nc.tensor.transpose(
                            rc_ps[:, v:v + 1],
                            r_sb[po:po + 1, fo:fo + 128],
                            ident_t[po:po + 1, po:po + 1])
                    nc.scalar.sqrt(std_col[:, csl], rc_ps[:, csl])
                    nc.vector.reciprocal(dis_col[:, csl], std_col[:, csl])
                    # y = YSCALE * dis * xw keeps fp8 out of the subnormals;
                    # the 1/YSCALE rides the relu scale (c1s)
                    nc.scalar.mul(diss_col[:, csl], dis_col[:, csl], YSCALE)
                    for jb in range(t * 8, t * 8 + 8):
                        ysl = y_t[jb // 2][:, jb % 2, :]
                        if jb % 2 == 0:
                            nc.vector.tensor_scalar_mul(
                                ysl, xw_t[jb][:], diss_col[:, jb:jb + 1])
                        else:
                            nc.scalar.mul(ysl, xw_t[jb][:],
                                          diss_col[:, jb:jb + 1])

            psXW_cm.__exit__(None, None, None)
            pXTh_cm.__exit__(None, None, None)

            if fold_scale:
                c1s_col = pCol.tile([128, JBN], F32, tag="c1s", bufs=1)
                nc.scalar.mul(c1s_col[:], dis_col[:], DSCALE / YSCALE)
            else:
                c1_col = pCol.tile([128, JBN], F32, tag="c1c", bufs=1)
                nc.vector.tensor_scalar_mul(c1_col[:], dis_col[:], ewc_t[:])
                nc.scalar.mul(c1_col[:], c1_col[:], 1.0 / YSCALE)

            # ---- phase M: one matmul group + relu/LN chain per 128-row
            # block; 8 PSUM banks rotate, freed by the relu read ----
            with tc.tile_pool(name="psMM", bufs=8, space="PSUM") as psMM, \
                 tc.tile_pool(name="pScr", bufs=9) as pScr, \
                 tc.tile_pool(name="pOut", bufs=5) as pOut:
                for ib in range(JBN):
                    lb = ib
                    ps = psMM.tile([128, 512], F32, tag="mm", name=f"mm{ib}")
                    # last block: split into o-halves so half-a's relu/hh/
                    # stats overlap half-b's matmuls, shortening the tail
                    halves = ((slice(0, 512),),) if ib != JBN - 1 else \
                        ((slice(0, 256),), (slice(256, 512),))
                    r = pScr.tile([128, D], F32, tag="scr", name=f"r{lb}")
                    # bf16 hh: halves the DVE/Pool cost of hh/bn_stats/t1
                    hh = pScr.tile([128, D], BF16, tag="scrh", name=f"hh{lb}")
                    st6 = pCol.tile([128, len(halves), 6], F32, tag="lnst",
                                    name=f"st{lb}")
                    for hi, (osl,) in enumerate(halves):
                        for j2 in range(JBN // 2):
                            nc.tensor.matmul(
                                ps[:, osl],
                                aT_t[j2][:, :, ib * 128:(ib + 1) * 128],
                                y_t[j2][:, :, osl], start=(j2 == 0),
                                stop=(j2 == JBN // 2 - 1),
                                perf_mode=mybir.MatmulPerfMode.DoubleRow)
                        if fold_scale:
                            nc.scalar.activation(
                                r[:, osl], ps[:, osl],
                                mybir.ActivationFunctionType.Relu,
                                scale=c1s_col[:, lb:lb + 1])
                        else:
                            tmp = pScr.tile([128, D], F32, tag="scr",
                                            name=f"tb{lb}_{hi}")
                            nc.vector.scalar_tensor_tensor(
                                tmp[:, osl], ps[:, osl], c1_col[:, lb:lb + 1],
                                bbT[:, osl], MUL, ADD)
                            nc.scalar.activation(
                                r[:, osl], tmp[:, osl],
                                mybir.ActivationFunctionType.Relu,
                                scale=DSCALE)
                        heng = nc.vector if lb % 2 == 0 else nc.gpsimd
                        heng.tensor_add(hh[:, osl], r[:, osl],
                                        x_t[lb][:, osl])
                        nc.vector.bn_stats(st6[:, hi, :], hh[:, osl])
                    mv = pCol.tile([128, 2], F32, tag="lnmv", name=f"mv{lb}")
                    nc.vector.bn_aggr(mv[:], st6[:])
                    stdt = pCol.tile([128, 1], F32, tag="lncol", name=f"sd{lb}")
                    nc.scalar.activation(
                        stdt[:], mv[:, 1:2], mybir.ActivationFunctionType.Sqrt,
                        bias=eps_t[:])
                    rstd = pCol.tile([128, 1], F32, tag="lncol", name=f"rs{lb}")
                    nc.vector.reciprocal(rstd[:], stdt[:])
                    eng1 = nc.gpsimd if lb % 2 == 0 else nc.vector
                    t1 = pOut.tile([128, D], F32, tag="o", name=f"t1{lb}")
                    eng1.tensor_scalar(t1[:], hh[:], mv[:, 0:1], rstd[:],
                                       SUB, MUL)
                    if ln_identity:
                        nc.scalar.dma_start(
                            out_d[lb * 128:(lb + 1) * 128, :], t1[:])
                    else:
                        tt = pScr.tile([128, D], F32, tag="scr", name=f"tt{lb}")
                        teng = nc.vector if lb % 2 == 0 else nc.gpsimd
                        teng.tensor_mul(tt[:], t1[:], stat_b["lnw_row"][:])
                        o_sb = pOut.tile([128, D], F32, tag="o", name=f"o{lb}")
                        nc.gpsimd.tensor_add(o_sb[:], tt[:],
                                             stat_b["lnb_row"][:])
                        nc.scalar.dma_start(
                            out_d[lb * 128:(lb + 1) * 128, :], o_sb[:])

    nc.compile()
    return nc


_NC_CACHE = {}


def _get_nc(fold_scale=True, ln_identity=True):
    key = (fold_scale, ln_identity)
    if key not in _NC_CACHE:
        _NC_CACHE[key] = _build_program(*key)
    return _NC_CACHE[key]


def kernel(x, adj, pad_mask, W, b, ln_w, ln_b, edge_weight):
    global LAST_RESULT
    x = np.asarray(x, dtype=np.float32)
    adj = np.asarray(adj, dtype=np.float32)
    pad_mask = np.asarray(pad_mask)
    W = np.asarray(W, dtype=np.float32)
    b = np.asarray(b, dtype=np.float32)
    ln_w = np.asarray(ln_w, dtype=np.float32)
    ln_b = np.asarray(ln_b, dtype=np.float32)
    ew = float(np.asarray(edge_weight).reshape(-1)[0])

    ln_identity = bool(np.all(ln_w == 1.0) and np.all(ln_b == 0.0))
    fold_scale = bool(np.all(b == 0.0) and ew >= 0.0)
    nc = _get_nc(fold_scale, ln_identity)

    ident = np.eye(128, dtype=np.float32)
    # fold_scale: ew commutes with W inside the relu argument, so fold it
    # into the weights host-side; deg/dis stay ew-free.
    w_eff = W.T * ew if fold_scale else W.T
    wt_h = np.ascontiguousarray(w_eff).astype(ml_dtypes.bfloat16)
    eye = np.eye(L, dtype=np.float32)

    in_maps = []
    for c in range(B):
        valid = (~pad_mask[c]).astype(np.float32)
        aT = (adj[c].T * valid[:, None]) * valid[None, :]
        aT += eye
        aT8 = aT.astype(ml_dtypes.float8_e4m3)
        aT8 = np.ascontiguousarray(
            aT8.reshape(8, 2, 128, L).transpose(0, 2, 1, 3).reshape(L // 2, 2 * L))
        im = {
            "ident": ident,
            "aT_p8": aT8,
            "xT_h": np.ascontiguousarray(x[c].T).astype(ml_dtypes.bfloat16),
            "x_in": np.ascontiguousarray(x[c]),
            "wt_h": wt_h,
        }
        if not fold_scale:
            im["ewc"] = np.full((128, 1), ew, dtype=np.float32)
            im["b_row"] = np.ascontiguousarray(b.reshape(1, D))
        if not ln_identity:
            im["lnw_row"] = np.ascontiguousarray(ln_w.reshape(1, D))
            im["lnb_row"] = np.ascontiguousarray(ln_b.reshape(1, D))
        in_maps.append(im)

    trace = os.environ.get("KERNEL_TRACE", "0") == "1"
    res = run_bass_kernel_spmd(nc, in_maps, core_ids=list(range(B)), trace=trace)
    LAST_RESULT = res
    out = np.stack([res.results[c]["out_t"] for c in range(B)], axis=0)
    return out


# revision 29
# speedup vs baseline: 1.0410x; 1.0410x over previous
"""GCN layer kernel for TRN2, data-parallel over batch across 8 NeuronCores.

The two matmuls of the layer compose linearly (the relu sits after both):
  out2[l,o] = dis_l * sum_j aT[j,l] * dis_j * (x @ W.T)[j,o]
so the device computes XW = x @ W.T first (bf16, from a host-transposed x,
during the adjacency DMA stream), then one big fp8 DoubleRow matmul against
the mask+self-loop-folded adjacency (host-prepared, fp8, paired-row layout
[128, 2, L] per j-tile) lands the pre-relu directly in [l, o] layout.

Phases:
  H: DMA wt/xT (bf16), then aT (fp8), then x (f32). PE: XW quarters
     (m-major, 3 PSUM banks) interleaved 3-matmuls-per-tile with the deg
     row-sum matvecs (fp8 DoubleRow vs a 16-wide ones stationary) so the PE
     rides the aT stream without idling.
  T: deg rows -> column layout via PE transposes, sqrt off PSUM,
     reciprocal; y = 16 * dis * XW cast to fp8 (the 16 keeps fp8 out of
     subnormals; 1/16 rides the relu scale).
  M: per 128-row block: 8 DoubleRow accumulation matmuls (j pairs), relu
     straight off PSUM with per-partition scale dis_l/(16*sqrt(D)), bf16
     residual hh = relu + x, bn_stats/bn_aggr moments, normalize, stream
     out. The last block runs in o-halves to shorten the serial tail.

Precision: adjacency and y in fp8e4m3, XW/W in bf16, accumulation in fp32
PSUM, LN in fp32 with bf16 hh. Measured rel err ~2-5e-3 vs the fp32
reference (gate: 2e-2).
"""
import os
import numpy as np
import ml_dtypes

import concourse.bacc as bacc
import concourse.tile as tile
import concourse.mybir as mybir
from concourse.bass_utils import run_bass_kernel_spmd

B, L, D = 8, 2048, 512
JBN = L // 128      # 16 row blocks (j tiles / l blocks)
NCH = L // 512      # 4 deg psum chunks of 512
DBN = D // 128      # 4 d-blocks
LN_EPS = 1e-5
DSCALE = float(D) ** -0.5
F32 = mybir.dt.float32
BF16 = mybir.dt.bfloat16
FP8 = mybir.dt.float8e4
YSCALE = 16.0
MUL = mybir.AluOpType.mult
ADD = mybir.AluOpType.add
SUB = mybir.AluOpType.subtract

LAST_RESULT = None  # BassKernelResults of the most recent run (for profiling)


def _build_program(fold_scale=True, ln_identity=True):
    """fold_scale: edge_weight folded into W host-side and bias == 0, so the
    relu collapses to an activation with per-partition scale dis_l*DSCALE.
    ln_identity: ln_w == 1, ln_b == 0."""
    nc = bacc.Bacc("TRN2", target_bir_lowering=False, debug=False)
    d = {}
    def di(name, shape, dt):
        d[name] = nc.dram_tensor(name, shape, dt, kind="ExternalInput").ap()
    di("ident", [128, 128], F32)
    di("aT_p8", [L // 2, 2 * L], FP8)
    di("xT_h", [D, L], BF16)
    di("x_in", [L, D], F32)
    di("wt_h", [D, D], BF16)
    if not fold_scale:
        di("ewc", [128, 1], F32)
        di("b_row", [1, D], F32)
    if not ln_identity:
        di("lnw_row", [1, D], F32)
        di("lnb_row", [1, D], F32)
    out_d = nc.dram_tensor("out_t", [L, D], F32, kind="ExternalOutput").ap()

    with tile.TileContext(nc) as tc:
        with tc.tile_pool(name="pA", bufs=JBN) as pA, \
             tc.tile_pool(name="pX", bufs=JBN) as pX, \
             tc.tile_pool(name="pY", bufs=JBN) as pY, \
             tc.tile_pool(name="pXW", bufs=JBN) as pXW, \
             tc.tile_pool(name="pW", bufs=DBN) as pW, \
             tc.tile_pool(name="pCol", bufs=12) as pCol, \
             tc.tile_pool(name="pSmall", bufs=1) as pSmall:

            # ---- persistent arrays ----
            aT_t = [pA.tile([128, 2, L], FP8, tag="aT", name=f"aT{j}")
                    for j in range(JBN // 2)]
            x_t = [pX.tile([128, D], F32, tag="x", name=f"x{j}") for j in range(JBN)]
            xw_t = [pXW.tile([128, D], BF16, tag="xw", name=f"xw{j}")
                    for j in range(JBN)]
            pXTh_cm = tc.tile_pool(name="pXTh", bufs=DBN)
            pXTh = pXTh_cm.__enter__()
            xTh_t = [pXTh.tile([128, L], BF16, tag="xTh", name=f"xTh{m}")
                     for m in range(DBN)]
            y_t = [pY.tile([128, 2, D], FP8, tag="y", name=f"y{j}")
                   for j in range(JBN // 2)]
            wt_t = [pW.tile([128, D], BF16, tag="wt", name=f"wt{k}")
                    for k in range(DBN)]
            eps_t = pSmall.tile([128, 1], F32, tag="eps")
            nc.vector.memset(eps_t[:], LN_EPS)
            # dual-fp8 ldweights needs 16B-aligned kk stride: 16 duplicate
            # ones columns; rows 1..15 of each deg psum output are ignored
            onesc_t = pSmall.tile([128, 2, 16], FP8, tag="onesc")
            nc.vector.memset(onesc_t[:], 1.0)
            # touch every activation function now so the Act table loads
            # happen during the DMA stream, not on the critical path later
            warm_t = pSmall.tile([128, 1], F32, tag="warm")
            nc.scalar.sqrt(warm_t[:], eps_t[:])
            nc.scalar.activation(warm_t[:], eps_t[:],
                                 mybir.ActivationFunctionType.Square)
            nc.scalar.activation(warm_t[:], eps_t[:],
                                 mybir.ActivationFunctionType.Relu)
            stat_b = {}

            # DMA order: ident, W, xT (for XW), aT (gates deg), x f32 last
            for k in range(DBN):
                nc.sync.dma_start(wt_t[k][:], d["wt_h"][k * 128:(k + 1) * 128, :])
                rsl = slice(k * 128, (k + 1) * 128)
                if k < 2:
                    # split: the first 512 columns feed XW quarter 0 sooner
                    nc.sync.dma_start(xTh_t[k][:, 0:512], d["xT_h"][rsl, 0:512])
                    nc.sync.dma_start(xTh_t[k][:, 512:L], d["xT_h"][rsl, 512:L])
                else:
                    nc.sync.dma_start(xTh_t[k][:], d["xT_h"][rsl, :])
            ident_t = pSmall.tile([128, 128], F32, tag="ident")
            nc.sync.dma_start(ident_t[:], d["ident"][:])
            for j2 in range(JBN // 2):
                nc.sync.dma_start(aT_t[j2][:],
                                  d["aT_p8"][j2 * 128:(j2 + 1) * 128, :])
            for jb in range(JBN):
                nc.sync.dma_start(x_t[jb][:], d["x_in"][jb * 128:(jb + 1) * 128, :])
            if not fold_scale:
                ewc_t = pSmall.tile([128, 1], F32, tag="ew")
                nc.scalar.dma_start(ewc_t[:], d["ewc"][:])
                b_r = pSmall.tile([1, D], F32, tag="brow")
                nc.scalar.dma_start(b_r[:], d["b_row"][:])
                bbT = pSmall.tile([128, D], F32, tag="bb")
                nc.gpsimd.partition_broadcast(bbT[:], b_r[:])
            if not ln_identity:
                rows = {}
                for nm in ("lnw_row", "lnb_row"):
                    r = pSmall.tile([1, D], F32, tag=nm, name=nm + "_t")
                    nc.scalar.dma_start(r[:], d[nm][:])
                    rows[nm] = r
                for nm in ("lnw_row", "lnb_row"):
                    t = pSmall.tile([128, D], F32, tag=nm + "b", name=nm + "_b")
                    nc.gpsimd.partition_broadcast(t[:], rows[nm][:])
                    stat_b[nm] = t

            # ---- phase H: XW = x @ W.T on PE from the host-transposed x,
            # m-major over jb-halves in 8 PSUM banks, while aT streams ----
            # XW emission is interleaved with the deg matvecs: quarter 0
            # runs before the aT stream lands; the remaining 48 XW matmuls
            # are doled out 3 per deg tile so the PE rides the stream with
            # no idle (per-tile slot: 4 deg matvecs + 3 XW matmuls ~= the
            # 1.46us tile DMA cadence).
            psXW_cm = tc.tile_pool(name="psXW", bufs=3, space="PSUM")
            psXW = psXW_cm.__enter__()

            def xw_instruction_stream():
                for q in range(4):
                    jbs = range(q * 4, q * 4 + 4)
                    xwp = {jb: psXW.tile([128, D], F32, tag="xwp",
                                         name=f"xwp{jb}") for jb in jbs}
                    for m in range(DBN):
                        for jb in jbs:
                            yield "mm", (lambda jb=jb, m=m, xwp=xwp:
                                nc.tensor.matmul(
                                    xwp[jb][:],
                                    xTh_t[m][:, jb * 128:(jb + 1) * 128],
                                    wt_t[m][:],
                                    start=(m == 0), stop=(m == DBN - 1)))
                    for jb in jbs:
                        if jb % 2 == 0:
                            yield "drain", (lambda jb=jb, xwp=xwp:
                                nc.vector.tensor_copy(xw_t[jb][:], xwp[jb][:]))
                        else:
                            yield "drain", (lambda jb=jb, xwp=xwp:
                                nc.scalar.copy(xw_t[jb][:], xwp[jb][:]))

            xw_stream = xw_instruction_stream()
            def emit_xw(k):
                n = 0
                for kind, op in xw_stream:
                    op()
                    if kind == "mm":
                        n += 1
                        if n >= k:
                            break

            emit_xw(20)  # quarter 0 + the head of quarter 1

            with tc.tile_pool(name="psDeg", bufs=4, space="PSUM") as psDeg, \
                 tc.tile_pool(name="psPT", bufs=1, space="PSUM") as psPT, \
                 tc.tile_pool(name="pTr", bufs=1) as pTr:
                # deg: 4 chunk accumulators, DoubleRow dst must start at
                # partition 0 so each chunk gets its own bank (row 0 used)
                deg_ps = [psDeg.tile([128, 512], F32, tag="deg",
                                     name=f"deg_ps{i}") for i in range(NCH)]
                for j2 in range(JBN // 2):
                    for n in range(NCH):
                        nc.tensor.matmul(
                            deg_ps[n][0:16, :],
                            onesc_t[:],
                            aT_t[j2][:, :, n * 512:(n + 1) * 512],
                            start=(j2 == 0), stop=(j2 == JBN // 2 - 1),
                            perf_mode=mybir.MatmulPerfMode.DoubleRow)
                    emit_xw(6)
                emit_xw(1000)  # drain any remaining XW work
                # ---- phase T: deg -> dis -> y, two-stage pipeline ----
                r_sb = pTr.tile([128, 1024], F32, tag="rsb")
                rc_ps = psPT.tile([128, JBN], F32, tag="rc")
                std_col = pCol.tile([128, JBN], F32, tag="stdc", bufs=1)
                dis_col = pCol.tile([128, JBN], F32, tag="disc", bufs=1)
                diss_col = pCol.tile([128, JBN], F32, tag="dissc", bufs=1)
                for t in range(2):
                    csl = slice(t * 8, t * 8 + 8)
                    eng_copy = (nc.vector.tensor_copy if t == 0
                                else nc.scalar.copy)
                    eng_copy(r_sb[0:1, t * 512:(t + 1) * 512],
                             deg_ps[2 * t][0:1, :])
                    eng_copy(r_sb[32:33, t * 512:(t + 1) * 512],
                             deg_ps[2 * t + 1][0:1, :])
                    for v in range(t * 8, t * 8 + 8):
                        n, c = v // 4, v % 4
                        po = 32 * (n % 2)
                        fo = (n // 2) * 512 + c * 128
                        nc.tensor.transpose(
                            rc_ps[:, v:v + 1],
                            r_sb[po:po + 1, fo:fo + 128],
                            ident_t[po:po + 1, po:po + 1])
                    nc.scalar.sqrt(std_col[:, csl], rc_ps[:, csl])
                    nc.vector.reciprocal(dis_col[:, csl], std_col[:, csl])
                    # y = YSCALE * dis * xw keeps fp8 out of the subnormals;
                    # the 1/YSCALE rides the relu scale (c1s)
                    nc.scalar.mul(diss_col[:, csl], dis_col[:, csl], YSCALE)
                    for jb in range(t * 8, t * 8 + 8):
                        ysl = y_t[jb // 2][:, jb % 2, :]
                        if jb % 2 == 0:
                            nc.vector.tensor_scalar_mul(
                                ysl, xw_t[jb][:], diss_col[:, jb:jb + 1])
                        else:
                            nc.scalar.mul(ysl, xw_t[jb][:],
                                          diss_col[:, jb:jb + 1])

            psXW_cm.__exit__(None, None, None)
            pXTh_cm.__exit__(None, None, None)

            if fold_scale:
                c1s_col = pCol.tile([128, JBN], F32, tag="c1s", bufs=1)
                nc.scalar.mul(c1s_col[:], dis_col[:], DSCALE / YSCALE)
            else:
                c1_col = pCol.tile([128, JBN], F32, tag="c1c", bufs=1)
                nc.vector.tensor_scalar_mul(c1_col[:], dis_col[:], ewc_t[:])
                nc.scalar.mul(c1_col[:], c1_col[:], 1.0 / YSCALE)

            # ---- phase M: one matmul group + relu/LN chain per 128-row
            # block; 8 PSUM banks rotate, freed by the relu read ----
            with tc.tile_pool(name="psMM", bufs=8, space="PSUM") as psMM, \
                 tc.tile_pool(name="pScr", bufs=9) as pScr, \
                 tc.tile_pool(name="pOut", bufs=5) as pOut:
                def emit_chain(lb, hh, st6):
                    mv = pCol.tile([128, 2], F32, tag="lnmv", name=f"mv{lb}")
                    nc.vector.bn_aggr(mv[:], st6[:])
                    stdt = pCol.tile([128, 1], F32, tag="lncol", name=f"sd{lb}")
                    nc.scalar.activation(
                        stdt[:], mv[:, 1:2], mybir.ActivationFunctionType.Sqrt,
                        bias=eps_t[:])
                    rstd = pCol.tile([128, 1], F32, tag="lncol", name=f"rs{lb}")
                    nc.vector.reciprocal(rstd[:], stdt[:])
                    eng1 = nc.gpsimd if lb % 2 == 0 else nc.vector
                    t1 = pOut.tile([128, D], F32, tag="o", name=f"t1{lb}")
                    eng1.tensor_scalar(t1[:], hh[:], mv[:, 0:1], rstd[:],
                                       SUB, MUL)
                    if ln_identity:
                        nc.scalar.dma_start(
                            out_d[lb * 128:(lb + 1) * 128, :], t1[:])
                    else:
                        tt = pScr.tile([128, D], F32, tag="scr", name=f"tt{lb}")
                        teng = nc.vector if lb % 2 == 0 else nc.gpsimd
                        teng.tensor_mul(tt[:], t1[:], stat_b["lnw_row"][:])
                        o_sb = pOut.tile([128, D], F32, tag="o", name=f"o{lb}")
                        nc.gpsimd.tensor_add(o_sb[:], tt[:],
                                             stat_b["lnb_row"][:])
                        nc.scalar.dma_start(
                            out_d[lb * 128:(lb + 1) * 128, :], o_sb[:])

                pending = []
                for ib in range(JBN):
                    lb = ib
                    # last block: split into o-halves with separate PSUM
                    # tiles so half-b's matmuls don't wait on half-a's relu
                    halves = ((slice(0, 512),),) if ib != JBN - 1 else \
                        ((slice(0, 256),), (slice(256, 512),))
                    ps_t = [psMM.tile([128, 512], F32, tag="mm",
                                      name=f"mm{ib}_{h}")
                            for h in range(len(halves))]
                    r = pScr.tile([128, D], F32, tag="scr", name=f"r{lb}")
                    hh = pScr.tile([128, D], BF16, tag="scrh", name=f"hh{lb}")
                    st6 = pCol.tile([128, len(halves), 6], F32, tag="lnst",
                                    name=f"st{lb}")
                    for hi, (osl,) in enumerate(halves):
                        ps = ps_t[hi]
                        for j2 in range(JBN // 2):
                            nc.tensor.matmul(
                                ps[:, osl],
                                aT_t[j2][:, :, ib * 128:(ib + 1) * 128],
                                y_t[j2][:, :, osl], start=(j2 == 0),
                                stop=(j2 == JBN // 2 - 1),
                                perf_mode=mybir.MatmulPerfMode.DoubleRow)
                        if fold_scale:
                            nc.scalar.activation(
                                r[:, osl], ps[:, osl],
                                mybir.ActivationFunctionType.Relu,
                                scale=c1s_col[:, lb:lb + 1])
                        else:
                            tmp = pScr.tile([128, D], F32, tag="scr",
                                            name=f"tb{lb}_{hi}")
                            nc.vector.scalar_tensor_tensor(
                                tmp[:, osl], ps[:, osl], c1_col[:, lb:lb + 1],
                                bbT[:, osl], MUL, ADD)
                            nc.scalar.activation(
                                r[:, osl], tmp[:, osl],
                                mybir.ActivationFunctionType.Relu,
                                scale=DSCALE)
                        heng = nc.vector if lb % 2 == 0 else nc.gpsimd
                        heng.tensor_add(hh[:, osl], r[:, osl],
                                        x_t[lb][:, osl])
                        nc.vector.bn_stats(st6[:, hi, :], hh[:, osl])
                    # stage the serial col-op chain one ib behind so the
                    # next relu is not queued behind this ib's sqrt on Act
                    pending.append((lb, hh, st6))
                    if len(pending) > 1:
                        emit_chain(*pending.pop(0))
                while pending:
                    emit_chain(*pending.pop(0))
    nc.compile()
    return nc


_NC_CACHE = {}


def _get_nc(fold_scale=True, ln_identity=True):
    key = (fold_scale, ln_identity)
    if key not in _NC_CACHE:
        _NC_CACHE[key] = _build_program(*key)
    return _NC_CACHE[key]


def kernel(x, adj, pad_mask, W, b, ln_w, ln_b, edge_weight):
    global LAST_RESULT
    x = np.asarray(x, dtype=np.float32)
    adj = np.asarray(adj, dtype=np.float32)
    pad_mask = np.asarray(pad_mask)
    W = np.asarray(W, dtype=np.float32)
    b = np.asarray(b, dtype=np.float32)
    ln_w = np.asarray(ln_w, dtype=np.float32)
    ln_b = np.asarray(ln_b, dtype=np.float32)
    ew = float(np.asarray(edge_weight).reshape(-1)[0])

    ln_identity = bool(np.all(ln_w == 1.0) and np.all(ln_b == 0.0))
    fold_scale = bool(np.all(b == 0.0) and ew >= 0.0)
    nc = _get_nc(fold_scale, ln_identity)

    ident = np.eye(128, dtype=np.float32)
    # fold_scale: ew commutes with W inside the relu argument, so fold it
    # into the weights host-side; deg/dis stay ew-free.
    w_eff = W.T * ew if fold_scale else W.T
    wt_h = np.ascontiguousarray(w_eff).astype(ml_dtypes.bfloat16)
    eye = np.eye(L, dtype=np.float32)

    in_maps = []
    for c in range(B):
        valid = (~pad_mask[c]).astype(np.float32)
        aT = (adj[c].T * valid[:, None]) * valid[None, :]
        aT += eye
        aT8 = aT.astype(ml_dtypes.float8_e4m3)
        aT8 = np.ascontiguousarray(
            aT8.reshape(8, 2, 128, L).transpose(0, 2, 1, 3).reshape(L // 2, 2 * L))
        im = {
            "ident": ident,
            "aT_p8": aT8,
            "xT_h": np.ascontiguousarray(x[c].T).astype(ml_dtypes.bfloat16),
            "x_in": np.ascontiguousarray(x[c]),
            "wt_h": wt_h,
        }
        if not fold_scale:
            im["ewc"] = np.full((128, 1), ew, dtype=np.float32)
            im["b_row"] = np.ascontiguousarray(b.reshape(1, D))
        if not ln_identity:
            im["lnw_row"] = np.ascontiguousarray(ln_w.reshape(1, D))
            im["lnb_row"] = np.ascontiguousarray(ln_b.reshape(1, D))
        in_maps.append(im)

    trace = os.environ.get("KERNEL_TRACE", "0") == "1"
    res = run_bass_kernel_spmd(nc, in_maps, core_ids=list(range(B)), trace=trace)
    LAST_RESULT = res
    out = np.stack([res.results[c]["out_t"] for c in range(B)], axis=0)
    return out


# revision 34
# speedup vs baseline: 1.1823x; 1.1357x over previous
"""GCN layer kernel for TRN2, data-parallel over batch across 8 NeuronCores.

The two matmuls of the layer compose linearly (the relu sits after both):
  out2[l,o] = dis_l * sum_j aT[j,l] * dis_j * (x @ W.T)[j,o]
so the device computes XW = x @ W.T first (bf16, from a host-transposed x,
during the adjacency DMA stream), then one big fp8 DoubleRow matmul against
the mask+self-loop-folded adjacency (host-prepared, fp8, paired-row layout
[128, 2, L] per j-tile) lands the pre-relu directly in [l, o] layout.

Phases:
  H: DMA wt/xT (bf16), then aT (fp8), then x (f32). PE: XW quarters
     (m-major, 3 PSUM banks) interleaved 3-matmuls-per-tile with the deg
     row-sum matvecs (fp8 DoubleRow vs a 16-wide ones stationary) so the PE
     rides the aT stream without idling.
  T: deg rows -> column layout via PE transposes, sqrt off PSUM,
     reciprocal; y = 16 * dis * XW cast to fp8 (the 16 keeps fp8 out of
     subnormals; 1/16 rides the relu scale).
  M: per 128-row block: 8 DoubleRow accumulation matmuls (j pairs), relu
     straight off PSUM with per-partition scale dis_l/(16*sqrt(D)), bf16
     residual hh = relu + x, bn_stats/bn_aggr moments, normalize, stream
     out. The last block runs in o-halves to shorten the serial tail.

Precision: adjacency and y in fp8e4m3, XW/W in bf16, accumulation in fp32
PSUM, LN in fp32 with bf16 hh. Measured rel err ~2-5e-3 vs the fp32
reference (gate: 2e-2).
"""
import os
import numpy as np
import ml_dtypes

import concourse.bacc as bacc
import concourse.tile as tile
import concourse.mybir as mybir
from concourse.bass_utils import run_bass_kernel_spmd

B, L, D = 8, 2048, 512
JBN = L // 128      # 16 row blocks (j tiles / l blocks)
NCH = L // 512      # 4 deg psum chunks of 512
DBN = D // 128      # 4 d-blocks
LN_EPS = 1e-5
DSCALE = float(D) ** -0.5
F32 = mybir.dt.float32
BF16 = mybir.dt.bfloat16
FP8 = mybir.dt.float8e4
YSCALE = 16.0
MUL = mybir.AluOpType.mult
ADD = mybir.AluOpType.add
SUB = mybir.AluOpType.subtract

LAST_RESULT = None  # BassKernelResults of the most recent run (for profiling)


def _build_program(fold_scale=True, ln_identity=True):
    """fold_scale: edge_weight folded into W host-side and bias == 0, so the
    relu collapses to an activation with per-partition scale dis_l*DSCALE.
    ln_identity: ln_w == 1, ln_b == 0."""
    nc = bacc.Bacc("TRN2", target_bir_lowering=False, debug=False)
    d = {}
    def di(name, shape, dt):
        d[name] = nc.dram_tensor(name, shape, dt, kind="ExternalInput").ap()
    di("ident", [128, 128], F32)
    di("aT_p8", [L // 2, 2 * L], FP8)
    di("xT_p8", [D // 2, 2 * L], FP8)
    di("x_in", [L, D], F32)
    di("wt_p8", [D // 2, 2 * D], FP8)
    if not fold_scale:
        di("ewc", [128, 1], F32)
        di("b_row", [1, D], F32)
    if not ln_identity:
        di("lnw_row", [1, D], F32)
        di("lnb_row", [1, D], F32)
    out_d = nc.dram_tensor("out_t", [L, D], F32, kind="ExternalOutput").ap()

    with tile.TileContext(nc) as tc:
        with tc.tile_pool(name="pA", bufs=JBN) as pA, \
             tc.tile_pool(name="pX", bufs=JBN) as pX, \
             tc.tile_pool(name="pY", bufs=JBN) as pY, \
             tc.tile_pool(name="pXW", bufs=JBN) as pXW, \
             tc.tile_pool(name="pW", bufs=DBN) as pW, \
             tc.tile_pool(name="pCol", bufs=12) as pCol, \
             tc.tile_pool(name="pSmall", bufs=1) as pSmall:

            # ---- persistent arrays ----
            aT_t = [pA.tile([128, 2, L], FP8, tag="aT", name=f"aT{j}")
                    for j in range(JBN // 2)]
            x_t = [pX.tile([128, D], F32, tag="x", name=f"x{j}") for j in range(JBN)]
            xw_t = [pXW.tile([128, D], BF16, tag="xw", name=f"xw{j}")
                    for j in range(JBN)]
            pXTh_cm = tc.tile_pool(name="pXTh", bufs=DBN // 2)
            pXTh = pXTh_cm.__enter__()
            xTh_t = [pXTh.tile([128, 2, L], FP8, tag="xTh", name=f"xTh{m}")
                     for m in range(DBN // 2)]
            y_t = [pY.tile([128, 2, D], FP8, tag="y", name=f"y{j}")
                   for j in range(JBN // 2)]
            wt_t = [pW.tile([128, 2, D], FP8, tag="wt", name=f"wt{k}")
                    for k in range(DBN // 2)]
            eps_t = pSmall.tile([128, 1], F32, tag="eps")
            nc.vector.memset(eps_t[:], LN_EPS)
            # dual-fp8 ldweights needs 16B-aligned kk stride: 16 duplicate
            # ones columns; rows 1..15 of each deg psum output are ignored
            onesc_t = pSmall.tile([128, 2, 16], FP8, tag="onesc")
            nc.vector.memset(onesc_t[:], 1.0)
            # touch every activation function now so the Act table loads
            # happen during the DMA stream, not on the critical path later
            warm_t = pSmall.tile([128, 1], F32, tag="warm")
            nc.scalar.sqrt(warm_t[:], eps_t[:])
            nc.scalar.activation(warm_t[:], eps_t[:],
                                 mybir.ActivationFunctionType.Square)
            nc.scalar.activation(warm_t[:], eps_t[:],
                                 mybir.ActivationFunctionType.Relu)
            stat_b = {}

            # DMA order: W, xT (fp8 pairs, for XW), aT (gates deg), x last
            for k in range(DBN // 2):
                rsl = slice(k * 128, (k + 1) * 128)
                nc.sync.dma_start(wt_t[k][:], d["wt_p8"][rsl, :])
                nc.sync.dma_start(xTh_t[k][:], d["xT_p8"][rsl, :])
            ident_t = pSmall.tile([128, 128], F32, tag="ident")
            nc.sync.dma_start(ident_t[:], d["ident"][:])
            for j2 in range(JBN // 2):
                nc.sync.dma_start(aT_t[j2][:],
                                  d["aT_p8"][j2 * 128:(j2 + 1) * 128, :])
            for jb in range(JBN):
                nc.sync.dma_start(x_t[jb][:], d["x_in"][jb * 128:(jb + 1) * 128, :])
            if not fold_scale:
                ewc_t = pSmall.tile([128, 1], F32, tag="ew")
                nc.scalar.dma_start(ewc_t[:], d["ewc"][:])
                b_r = pSmall.tile([1, D], F32, tag="brow")
                nc.scalar.dma_start(b_r[:], d["b_row"][:])
                bbT = pSmall.tile([128, D], F32, tag="bb")
                nc.gpsimd.partition_broadcast(bbT[:], b_r[:])
            if not ln_identity:
                rows = {}
                for nm in ("lnw_row", "lnb_row"):
                    r = pSmall.tile([1, D], F32, tag=nm, name=nm + "_t")
                    nc.scalar.dma_start(r[:], d[nm][:])
                    rows[nm] = r
                for nm in ("lnw_row", "lnb_row"):
                    t = pSmall.tile([128, D], F32, tag=nm + "b", name=nm + "_b")
                    nc.gpsimd.partition_broadcast(t[:], rows[nm][:])
                    stat_b[nm] = t

            # ---- phase H: XW = x @ W.T on PE from the host-transposed x,
            # m-major over jb-halves in 8 PSUM banks, while aT streams ----
            # XW emission is interleaved with the deg matvecs: quarter 0
            # runs before the aT stream lands; the remaining 48 XW matmuls
            # are doled out 3 per deg tile so the PE rides the stream with
            # no idle (per-tile slot: 4 deg matvecs + 3 XW matmuls ~= the
            # 1.46us tile DMA cadence).
            psXW_cm = tc.tile_pool(name="psXW", bufs=3, space="PSUM")
            psXW = psXW_cm.__enter__()

            def xw_instruction_stream():
                for q in range(4):
                    jbs = range(q * 4, q * 4 + 4)
                    xwp = {jb: psXW.tile([128, D], F32, tag="xwp",
                                         name=f"xwp{jb}") for jb in jbs}
                    for m in range(DBN // 2):
                        for jb in jbs:
                            yield "mm", (lambda jb=jb, m=m, xwp=xwp:
                                nc.tensor.matmul(
                                    xwp[jb][:],
                                    xTh_t[m][:, :, jb * 128:(jb + 1) * 128],
                                    wt_t[m][:],
                                    start=(m == 0), stop=(m == DBN // 2 - 1),
                                    perf_mode=mybir.MatmulPerfMode.DoubleRow))
                    for jb in jbs:
                        if jb % 2 == 0:
                            yield "drain", (lambda jb=jb, xwp=xwp:
                                nc.vector.tensor_copy(xw_t[jb][:], xwp[jb][:]))
                        else:
                            yield "drain", (lambda jb=jb, xwp=xwp:
                                nc.scalar.copy(xw_t[jb][:], xwp[jb][:]))

            xw_stream = xw_instruction_stream()
            def emit_xw(k):
                n = 0
                for kind, op in xw_stream:
                    op()
                    if kind == "mm":
                        n += 1
                        if n >= k:
                            break

            emit_xw(20)  # quarter 0 + the head of quarter 1

            with tc.tile_pool(name="psDeg", bufs=4, space="PSUM") as psDeg, \
                 tc.tile_pool(name="psPT", bufs=1, space="PSUM") as psPT, \
                 tc.tile_pool(name="pTr", bufs=1) as pTr:
                # deg: 4 chunk accumulators, DoubleRow dst must start at
                # partition 0 so each chunk gets its own bank (row 0 used)
                deg_ps = [psDeg.tile([128, 512], F32, tag="deg",
                                     name=f"deg_ps{i}") for i in range(NCH)]
                for j2 in range(JBN // 2):
                    for n in range(NCH):
                        nc.tensor.matmul(
                            deg_ps[n][0:16, :],
                            onesc_t[:],
                            aT_t[j2][:, :, n * 512:(n + 1) * 512],
                            start=(j2 == 0), stop=(j2 == JBN // 2 - 1),
                            perf_mode=mybir.MatmulPerfMode.DoubleRow)
                    emit_xw(6)
                emit_xw(1000)  # drain any remaining XW work
                # ---- phase T: deg -> dis -> y, two-stage pipeline ----
                r_sb = pTr.tile([128, 1024], F32, tag="rsb")
                rc_ps = psPT.tile([128, JBN], F32, tag="rc")
                std_col = pCol.tile([128, JBN], F32, tag="stdc", bufs=1)
                dis_col = pCol.tile([128, JBN], F32, tag="disc", bufs=1)
                diss_col = pCol.tile([128, JBN], F32, tag="dissc", bufs=1)
                for t in range(2):
                    csl = slice(t * 8, t * 8 + 8)
                    eng_copy = (nc.vector.tensor_copy if t == 0
                                else nc.scalar.copy)
                    eng_copy(r_sb[0:1, t * 512:(t + 1) * 512],
                             deg_ps[2 * t][0:1, :])
                    eng_copy(r_sb[32:33, t * 512:(t + 1) * 512],
                             deg_ps[2 * t + 1][0:1, :])
                    for v in range(t * 8, t * 8 + 8):
                        n, c = v // 4, v % 4
                        po = 32 * (n % 2)
                        fo = (n // 2) * 512 + c * 128
                        nc.tensor.transpose(
                            rc_ps[:, v:v + 1],
                            r_sb[po:po + 1, fo:fo + 128],
                            ident_t[po:po + 1, po:po + 1])
                    nc.scalar.sqrt(std_col[:, csl], rc_ps[:, csl])
                    nc.vector.reciprocal(dis_col[:, csl], std_col[:, csl])
                    # y = YSCALE * dis * xw keeps fp8 out of the subnormals;
                    # the 1/YSCALE rides the relu scale (c1s)
                    nc.scalar.mul(diss_col[:, csl], dis_col[:, csl], YSCALE)
                    for jb in range(t * 8, t * 8 + 8):
                        ysl = y_t[jb // 2][:, jb % 2, :]
                        if jb % 2 == 0:
                            nc.vector.tensor_scalar_mul(
                                ysl, xw_t[jb][:], diss_col[:, jb:jb + 1])
                        else:
                            nc.scalar.mul(ysl, xw_t[jb][:],
                                          diss_col[:, jb:jb + 1])

            psXW_cm.__exit__(None, None, None)
            pXTh_cm.__exit__(None, None, None)

            if fold_scale:
                c1s_col = pCol.tile([128, JBN], F32, tag="c1s", bufs=1)
                nc.scalar.mul(c1s_col[:], dis_col[:], DSCALE / YSCALE)
            else:
                c1_col = pCol.tile([128, JBN], F32, tag="c1c", bufs=1)
                nc.vector.tensor_scalar_mul(c1_col[:], dis_col[:], ewc_t[:])
                nc.scalar.mul(c1_col[:], c1_col[:], 1.0 / YSCALE)

            # ---- phase M: one matmul group + relu/LN chain per 128-row
            # block; 8 PSUM banks rotate, freed by the relu read ----
            with tc.tile_pool(name="psMM", bufs=8, space="PSUM") as psMM, \
                 tc.tile_pool(name="pScr", bufs=9) as pScr, \
                 tc.tile_pool(name="pOut", bufs=5) as pOut:
                def emit_chain(lb, hh, st6):
                    mv = pCol.tile([128, 2], F32, tag="lnmv", name=f"mv{lb}")
                    nc.vector.bn_aggr(mv[:], st6[:])
                    stdt = pCol.tile([128, 1], F32, tag="lncol", name=f"sd{lb}")
                    nc.scalar.activation(
                        stdt[:], mv[:, 1:2], mybir.ActivationFunctionType.Sqrt,
                        bias=eps_t[:])
                    rstd = pCol.tile([128, 1], F32, tag="lncol", name=f"rs{lb}")
                    nc.vector.reciprocal(rstd[:], stdt[:])
                    eng1 = nc.gpsimd if lb % 2 == 0 else nc.vector
                    t1 = pOut.tile([128, D], F32, tag="o", name=f"t1{lb}")
                    eng1.tensor_scalar(t1[:], hh[:], mv[:, 0:1], rstd[:],
                                       SUB, MUL)
                    if ln_identity:
                        nc.scalar.dma_start(
                            out_d[lb * 128:(lb + 1) * 128, :], t1[:])
                    else:
                        tt = pScr.tile([128, D], F32, tag="scr", name=f"tt{lb}")
                        teng = nc.vector if lb % 2 == 0 else nc.gpsimd
                        teng.tensor_mul(tt[:], t1[:], stat_b["lnw_row"][:])
                        o_sb = pOut.tile([128, D], F32, tag="o", name=f"o{lb}")
                        nc.gpsimd.tensor_add(o_sb[:], tt[:],
                                             stat_b["lnb_row"][:])
                        nc.scalar.dma_start(
                            out_d[lb * 128:(lb + 1) * 128, :], o_sb[:])

                pending = []
                for ib in range(JBN):
                    lb = ib
                    # last block: split into o-halves with separate PSUM
                    # tiles so half-b's matmuls don't wait on half-a's relu
                    halves = ((slice(0, 512),),) if ib != JBN - 1 else \
                        ((slice(0, 256),), (slice(256, 512),))
                    ps_t = [psMM.tile([128, 512], F32, tag="mm",
                                      name=f"mm{ib}_{h}")
                            for h in range(len(halves))]
                    r = pScr.tile([128, D], F32, tag="scr", name=f"r{lb}")
                    hh = pScr.tile([128, D], BF16, tag="scrh", name=f"hh{lb}")
                    st6 = pCol.tile([128, len(halves), 6], F32, tag="lnst",
                                    name=f"st{lb}")
                    for hi, (osl,) in enumerate(halves):
                        ps = ps_t[hi]
                        for j2 in range(JBN // 2):
                            nc.tensor.matmul(
                                ps[:, osl],
                                aT_t[j2][:, :, ib * 128:(ib + 1) * 128],
                                y_t[j2][:, :, osl], start=(j2 == 0),
                                stop=(j2 == JBN // 2 - 1),
                                perf_mode=mybir.MatmulPerfMode.DoubleRow)
                        if fold_scale:
                            nc.scalar.activation(
                                r[:, osl], ps[:, osl],
                                mybir.ActivationFunctionType.Relu,
                                scale=c1s_col[:, lb:lb + 1])
                        else:
                            tmp = pScr.tile([128, D], F32, tag="scr",
                                            name=f"tb{lb}_{hi}")
                            nc.vector.scalar_tensor_tensor(
                                tmp[:, osl], ps[:, osl], c1_col[:, lb:lb + 1],
                                bbT[:, osl], MUL, ADD)
                            nc.scalar.activation(
                                r[:, osl], tmp[:, osl],
                                mybir.ActivationFunctionType.Relu,
                                scale=DSCALE)
                        heng = nc.vector if lb % 2 == 0 else nc.gpsimd
                        heng.tensor_add(hh[:, osl], r[:, osl],
                                        x_t[lb][:, osl])
                        nc.vector.bn_stats(st6[:, hi, :], hh[:, osl])
                    # stage the serial col-op chain one ib behind so the
                    # next relu is not queued behind this ib's sqrt on Act
                    pending.append((lb, hh, st6))
                    if len(pending) > 1:
                        emit_chain(*pending.pop(0))
                while pending:
                    emit_chain(*pending.pop(0))
    nc.compile()
    return nc


_NC_CACHE = {}


def _get_nc(fold_scale=True, ln_identity=True):
    key = (fold_scale, ln_identity)
    if key not in _NC_CACHE:
        _NC_CACHE[key] = _build_program(*key)
    return _NC_CACHE[key]


def kernel(x, adj, pad_mask, W, b, ln_w, ln_b, edge_weight):
    global LAST_RESULT
    x = np.asarray(x, dtype=np.float32)
    adj = np.asarray(adj, dtype=np.float32)
    pad_mask = np.asarray(pad_mask)
    W = np.asarray(W, dtype=np.float32)
    b = np.asarray(b, dtype=np.float32)
    ln_w = np.asarray(ln_w, dtype=np.float32)
    ln_b = np.asarray(ln_b, dtype=np.float32)
    ew = float(np.asarray(edge_weight).reshape(-1)[0])

    ln_identity = bool(np.all(ln_w == 1.0) and np.all(ln_b == 0.0))
    fold_scale = bool(np.all(b == 0.0) and ew >= 0.0)
    nc = _get_nc(fold_scale, ln_identity)

    ident = np.eye(128, dtype=np.float32)
    # fold_scale: ew commutes with W inside the relu argument, so fold it
    # into the weights host-side; deg/dis stay ew-free.
    w_eff = W.T * ew if fold_scale else W.T
    wt8 = w_eff.astype(ml_dtypes.float8_e4m3)
    wt_p8 = np.ascontiguousarray(
        wt8.reshape(2, 2, 128, D).transpose(0, 2, 1, 3).reshape(D // 2, 2 * D))
    eye = np.eye(L, dtype=np.float32)

    in_maps = []
    for c in range(B):
        valid = (~pad_mask[c]).astype(np.float32)
        aT = (adj[c].T * valid[:, None]) * valid[None, :]
        aT += eye
        aT8 = aT.astype(ml_dtypes.float8_e4m3)
        aT8 = np.ascontiguousarray(
            aT8.reshape(8, 2, 128, L).transpose(0, 2, 1, 3).reshape(L // 2, 2 * L))
        im = {
            "ident": ident,
            "aT_p8": aT8,
            "xT_p8": np.ascontiguousarray(
                x[c].T.astype(ml_dtypes.float8_e4m3)
                .reshape(2, 2, 128, L).transpose(0, 2, 1, 3)
                .reshape(D // 2, 2 * L)),
            "x_in": np.ascontiguousarray(x[c]),
            "wt_p8": wt_p8,
        }
        if not fold_scale:
            im["ewc"] = np.full((128, 1), ew, dtype=np.float32)
            im["b_row"] = np.ascontiguousarray(b.reshape(1, D))
        if not ln_identity:
            im["lnw_row"] = np.ascontiguousarray(ln_w.reshape(1, D))
            im["lnb_row"] = np.ascontiguousarray(ln_b.reshape(1, D))
        in_maps.append(im)

    trace = os.environ.get("KERNEL_TRACE", "0") == "1"
    res = run_bass_kernel_spmd(nc, in_maps, core_ids=list(range(B)), trace=trace)
    LAST_RESULT = res
    out = np.stack([res.results[c]["out_t"] for c in range(B)], axis=0)
    return out
